# revision 11
# baseline (speedup 1.0000x reference)
"""Bezier-to-image Gaussian splat kernel for Trainium2 (8 NeuronCores).

Reference computation (per sample b of 256):
    T = warped cubic Bernstein basis (30, 4)
    points = einsum('nk,blkc->blnc', T, x.reshape(B,160,4,2))   # (B,160,30,2)
    gx[b,l,i,n] = exp(-(i/60 - X[b,l,n])^2 / 2e-4)
    out[b,i,j]  = min(sum_{l,n} gx[b,l,i,n]*gy[b,l,j,n], 1)     # (B,60,60)

Strategy: pure data parallel, 32 samples per core.  Per sample the 4800
bezier points sit in a [128, 40] layout (partition p = 32*lg + n, chunk c
selects curve l = 4c + lg; rows n in {30,31} of each 32-strip are dead).

The Scalar engine's exp throughput (1 elem/cycle) is the roofline:
2 sides x 40 chunks x 60 cells = 4800 exps per sample (~4.3us).  Everything
else is arranged to stay under that pace:

  * x-side distances d = i - 60X are built by one batched DVE
    tensor_tensor with broadcast APs ([128, 2400] fp16, ~2.7us).
  * y-side distances live RESIDENT IN PSUM (5 banks, [128, 2400] fp32):
    initialised once as iota via a ones^T @ iota matmul, then updated per
    sample by 4 column-tiled delta-matmuls D += tsc @ (ct[b-1]-ct[b])
    with the rhs broadcast along the 60 cells (stride-0 AP).  The Scalar
    engine evaluates Derivative_Erf straight out of PSUM, so the y side
    costs the DVE nearly nothing (one tiny [5,160] subtract).
  * the 60x60 image accumulates on the Tensor engine as
    sum_c GxT_c^T @ GyT_c into one PSUM bank, then is DMA'd to HBM raw;
    the final min(scale*img, 1) runs on the host over the gathered
    output (trivial compared to device work).
"""

import math

import numpy as np
import orjson

import bass_rust
import concourse.bass as bass
import concourse.mybir as mybir
import concourse.tile as tile
from concourse.bass_utils import run_bass_kernel_spmd

B, L, N, W = 256, 160, 30, 60
NCORES = 8
BC = B // NCORES          # samples per core
ALPHA = 2e-4
KEXP = 1.0 / (W * W * ALPHA)          # exponent scale in cell units: 1/0.72
SDERF = math.sqrt(KEXP)               # Derivative_Erf input scale
DERF_FIX = math.pi / 4.0              # undo (2/sqrt(pi))^2 from Derivative_Erf
CHUNKS = 40                           # 4 curves x 30 samples per chunk
PTS = 128                             # chunk partition dim: p = 32*lg + n
CW = 60                               # cells per axis
FD = CHUNKS * CW                      # 2400: per-side band free dim
R_HOLE = -60.0                        # dead-row r -> d in [60,119] -> g = 0
BANKC = 8                             # chunks per PSUM-bank chunk of D (8*60*4B = 1920B)
NBANKCH = CHUNKS // BANKC             # 5 delta-matmul column groups
BANKF = 512                           # f32 slots per PSUM bank (2KB); 480 used + 32 pad
DFD = NBANKCH * BANKF                 # 2560: D_y free dim incl. per-bank padding

LAST_RESULTS = None  # test harness reads profiling info from here


def _basis_T() -> np.ndarray:
    t = np.arange(N, dtype=np.float32) / np.float32(N)
    t = 2 * t**3 - 3 * t**2 + 2 * t
    t_3_0 = t**3
    t_2_1 = t**2 - t_3_0
    t_1_2 = t_3_0 - 2 * t**2 + t
    t_0_3 = (1 - t) ** 3
    return np.stack([t_3_0, 3 * t_2_1, 3 * t_1_2, t_0_3], axis=1).astype(np.float32)


def _legalize_waits(nc, max_waits: int = 1):
    """Walrus rejects engine instructions carrying more than ~1 sync wait
    ("Too many sync wait commands").  Hoist excess waits onto same-engine
    Drain instructions inserted immediately before the offender."""
    js = orjson.loads(mybir.module_to_json_bytes(nc.m))
    ctr = 0
    for f in js["functions"]:
        for bb in f["blocks"]:
            out = []
            changed = False
            for inst in bb["instructions"]:
                si = inst.get("sync_info")
                waits = si.get("on_wait") if si else None
                if waits and len(waits) > max_waits:
                    keep = waits[:max_waits]
                    for w in waits[max_waits:]:
                        ctr += 1
                        out.append({
                            "debug": inst.get("debug", 0),
                            "engine": inst["engine"],
                            "ins": [], "outs": [],
                            "name": f"waitfix-{ctr}",
                            "opcode": "Drain",
                            "sync_info": {"on_update": [], "on_wait": [w]},
                        })
                    si["on_wait"] = keep
                    changed = True
                out.append(inst)
            if changed:
                bb["instructions"] = out
    if ctr:
        nc.m = bass_rust.module_from_json_bytes(orjson.dumps(js))
    return ctr


def build_program(legalize: bool = True):
    f32 = mybir.dt.float32
    f16 = mybir.dt.float16

    nc = bass.Bass("TRN2", target_bir_lowering=False, debug=False)

    x_t = nc.dram_tensor("x", [BC, L, 8], f32, kind="ExternalInput")
    y_t = nc.dram_tensor("y", [BC, W, W], f32, kind="ExternalOutput")

    # (5, 32) stationary operand: r[m] = sum_k TscT[k,m]*ctrl[k] = 60*X for
    # m<30; row 4 contracts against a constant-ones row so the two dead
    # partitions of each 32-strip get r = R_HOLE (x side) / D += 60 once at
    # sample 0 (y side, where the ones-row delta is -1 at b=0 and 0 after).
    tsc_np = np.zeros((5, 32), dtype=np.float32)
    tsc_np[:4, :N] = (W * _basis_T()).T
    tsc_np[4, N:] = R_HOLE
    tsc_d = nc.inline_tensor(tsc_np, name="tscT")
    iota_np = np.tile(np.arange(CW, dtype=np.float16), (PTS, 1))  # (128, 60)
    iota_d = nc.inline_tensor(iota_np, name="iota60")
    # y-side iota, bank-padded: each 512-slot bank holds 8 chunks of 60 cells
    # plus 32 pad slots pinned at 60 (-> g = 0, deltas never touch them).
    iotay_np = np.full((1, DFD), 60.0, dtype=np.float32)
    for j in range(NBANKCH):
        iotay_np[0, BANKF * j : BANKF * j + BANKC * CW] = np.tile(
            np.arange(CW, dtype=np.float32), BANKC
        )
    iotay_d = nc.inline_tensor(iotay_np, name="iotaY")
    ones_np = np.ones((1, PTS), dtype=np.float32)
    ones_d = nc.inline_tensor(ones_np, name="onesRow")

    with tile.TileContext(nc) as tc, tc.tile_pool(name="const", bufs=1) as cpool, \
            tc.tile_pool(name="ctrl", bufs=1) as ctrl_pool, \
            tc.tile_pool(name="rwork", bufs=2) as rpool, \
            tc.tile_pool(name="dd", bufs=2) as dd_pool, \
            tc.tile_pool(name="gg", bufs=2) as gg_pool, \
            tc.tile_pool(name="dps", bufs=1, space="PSUM") as d_pool, \
            tc.tile_pool(name="rpsum", bufs=1, space="PSUM") as rps_pool, \
            tc.tile_pool(name="imgpsum", bufs=2, space="PSUM") as img_pool:

        # Prologue: DMA loads land in staging tiles; DVE copies them into the
        # tiles PE reads (PE LDWEIGHTS tolerates very few sync waits).
        tsc0 = cpool.tile([5, 32], f32, tag="tsc0")
        nc.sync.dma_start(tsc0[:], tsc_d.ap())
        tsc = cpool.tile([5, 32], f32, tag="tsc")
        nc.vector.tensor_copy(tsc[:], tsc0[:])
        iot = cpool.tile([PTS, CW], f16, tag="iota")
        nc.sync.dma_start(iot[:], iota_d.ap())
        ioy0 = cpool.tile([1, DFD], f32, tag="ioy0")
        nc.sync.dma_start(ioy0[:], iotay_d.ap())
        ioy = cpool.tile([1, DFD], f32, tag="ioy")
        nc.vector.tensor_copy(ioy[:], ioy0[:])
        one0 = cpool.tile([1, PTS], f32, tag="one0")
        nc.sync.dma_start(one0[:], ones_d.ap())
        onesr = cpool.tile([1, PTS], f32, tag="ones")
        nc.vector.tensor_copy(onesr[:], one0[:])

        # control points: partition k (4) + ones row (4), free = (b, l, coord).
        # Loaded in groups of 4 samples, alternating between the two HWDGE
        # queues (qSP / qAct) so the descriptor-heavy strided loads overlap.
        GRP = 4
        ct = ctrl_pool.tile([5, BC * 2 * L], f32, tag="ct")
        nc.vector.memset(ct[:], 1.0)          # row 4 stays all-ones
        gsz = GRP * 2 * L
        for g in range(BC // GRP):
            ct0 = rpool.tile([4, gsz], f32, tag="ct0")
            eng = nc.sync if g % 2 == 0 else nc.scalar
            eng.dma_start(
                ct0[:].rearrange("k (b l c) -> k b l c", b=GRP, c=2),
                x_t.ap()[g * GRP : (g + 1) * GRP]
                .rearrange("b l (k c) -> k b l c", k=4),
            )
            nc.vector.tensor_copy(ct[0:4, g * gsz : (g + 1) * gsz], ct0[:])
        ct_v = ct[:].rearrange("k (b c g co) -> k b c g co", b=BC, c=CHUNKS, co=2)

        # ---- D_y: iota - 60*Y resident in PSUM (5 banks) ----
        dY = d_pool.tile([PTS, DFD], f32, tag="dY")
        for j in range(NBANKCH):
            nc.tensor.matmul(
                dY[:, BANKF * j : BANKF * (j + 1)],
                lhsT=onesr[:],
                rhs=ioy[:, BANKF * j : BANKF * (j + 1)],
                start=True,
                stop=True,
            )

        def emit_dct(b):
            """dct = ct_y[b-1] - ct_y[b]  (b=0: 0 - ct_y[0]); [5, (c, g)]."""
            dct = rpool.tile([5, CHUNKS * 4], f32, tag="dct")
            cur = ct_v[:, b : b + 1, :, :, 1:2]
            if b == 0:
                nc.vector.tensor_scalar_mul(dct[:], cur, -1.0)
            else:
                prev = ct_v[:, b - 1 : b, :, :, 1:2]
                nc.vector.tensor_tensor(
                    dct[:], prev, cur, mybir.AluOpType.subtract
                )
            return dct

        def emit_delta_mms(dct):
            """D_y += tsc @ dct, rhs broadcast along the 60 cells."""
            dct_v = dct[:].rearrange("k (c g o) -> k c g o", g=4, o=1)
            for g in range(4):
                for j in range(NBANKCH):
                    nc.tensor.matmul(
                        dY[32 * g : 32 * g + 32,
                           BANKF * j : BANKF * j + BANKC * CW],
                        lhsT=tsc[:],
                        rhs=dct_v[:, BANKC * j : BANKC * (j + 1), g : g + 1, :]
                        .broadcast_to([5, BANKC, 1, CW]),
                        start=False,
                        stop=True,
                        tile_position=(0, 32 * g),
                    )

        def emit_rx(b):
            """x-side r = 60*X into r_ps, copy to SBUF, build dd_x bands."""
            r_ps = rps_pool.tile([PTS, CHUNKS], f32, tag="rps")
            for g in range(4):
                nc.tensor.matmul(
                    r_ps[32 * g : 32 * g + 32, :],
                    lhsT=tsc[:],
                    rhs=ct_v[:, b : b + 1, :, g : g + 1, 0:1],
                    start=True,
                    stop=True,
                    tile_position=(0, 32 * g),
                )
            r_sb = rpool.tile([PTS, CHUNKS], f32, tag="rsb")
            nc.vector.tensor_copy(r_sb[:], r_ps[:])
            dd = dd_pool.tile([PTS, FD], f16, tag="dd")
            nc.vector.tensor_tensor(
                dd[:].rearrange("p (c w) -> p c w", w=CW),
                iot[:].rearrange("p (o w) -> p o w", o=1).broadcast_to(
                    [PTS, CHUNKS, CW]
                ),
                r_sb[:].rearrange("p (c o) -> p c o", o=1).broadcast_to(
                    [PTS, CHUNKS, CW]
                ),
                mybir.AluOpType.subtract,
            )
            return dd

        # ---- sample 0 bootstrap ----
        dd_cur = emit_rx(0)
        dct0 = emit_dct(0)
        emit_delta_mms(dct0)

        for b in range(BC):
            # gx from SBUF fp16 bands, gy straight from PSUM
            gg_x = gg_pool.tile([PTS, FD], f16, tag="ggx")
            nc.scalar.activation(
                gg_x[:], dd_cur[:],
                mybir.ActivationFunctionType.Derivative_Erf,
                bias=0.0, scale=SDERF,
            )
            gg_y = gg_pool.tile([PTS, DFD], f16, tag="ggy")
            nc.scalar.activation(
                gg_y[:], dY[:],
                mybir.ActivationFunctionType.Derivative_Erf,
                bias=0.0, scale=SDERF,
            )

            if b + 1 < BC:
                # x side of b+1 (PE r-matmuls early, DVE band build under gy)
                dd_cur = emit_rx(b + 1)
                # y side of b+1: delta-matmuls wait on gy(b)'s PSUM read
                dct = emit_dct(b + 1)
                emit_delta_mms(dct)

            # ---- image accumulation: sum_c GxT_c^T @ GyT_c ----
            img = img_pool.tile([W, W], f32, tag="img")
            for c in range(CHUNKS):
                yoff = BANKF * (c // BANKC) + CW * (c % BANKC)
                nc.tensor.matmul(
                    img[:],
                    lhsT=gg_x[:, CW * c : CW * c + W],
                    rhs=gg_y[:, yoff : yoff + W],
                    start=(c == 0),
                    stop=(c == CHUNKS - 1),
                )
            # min(scale*img, 1) -> SBUF staging, then out
            osb = rpool.tile([W, W], f32, tag="osb")
            nc.vector.tensor_scalar(
                osb[:], img[:], DERF_FIX, 1.0,
                mybir.AluOpType.mult, mybir.AluOpType.min,
            )
            nc.sync.dma_start(y_t.ap()[b : b + 1], osb[:])

    if legalize:
        _legalize_waits(nc)
    return nc


_PROGRAM = None


def kernel(x: np.ndarray, _trace: bool = False) -> np.ndarray:
    global _PROGRAM, LAST_RESULTS
    assert x.shape == (B, L, 8) and x.dtype == np.float32, (x.shape, x.dtype)
    if _PROGRAM is None:
        _PROGRAM = build_program()
    nc = _PROGRAM
    shards = np.split(np.ascontiguousarray(x), NCORES, axis=0)
    in_maps = [{"x": s} for s in shards]
    res = run_bass_kernel_spmd(nc, in_maps, list(range(NCORES)), trace=_trace)
    LAST_RESULTS = res
    return np.concatenate([res.results[i]["y"] for i in range(NCORES)], axis=0)


# revision 13
# speedup vs baseline: 2.4632x; 2.4632x over previous
"""Bezier-to-image Gaussian splat kernel for Trainium2 (8 NeuronCores).

Reference computation (per sample b of 256):
    T = warped cubic Bernstein basis (30, 4)
    points = einsum('nk,blkc->blnc', T, x.reshape(B,160,4,2))   # (B,160,30,2)
    gx[b,l,i,n] = exp(-(i/60 - X[b,l,n])^2 / 2e-4)
    out[b,i,j]  = min(sum_{l,n} gx[b,l,i,n]*gy[b,l,j,n], 1)     # (B,60,60)

Strategy: pure data parallel, 32 samples per core.  Per sample the 4800
bezier points are processed in 40 chunks of 120 points (partition dim);
d[p,i] = i - 60*X_p is built by ONE batched DVE tensor_tensor against a
constant iota row (fp16; the broadcast APs cap it at 1x mode, which is
still the cheapest exact option on this hardware), the Gaussian is
evaluated on the Scalar engine (Derivative_Erf LUT = 2/sqrt(pi)*exp(-x^2)
in a single batched pass), and the 60x60 image is accumulated on the
Tensor engine as sum_c GxT_c^T @ GyT_c into one PSUM bank.

vs the original baseline: the r PSUM->SBUF copy runs on the Scalar engine
(DVE is the pace-setter at ~5.4us/sample), the input ctrl points stream in
8 groups alternating between the two HWDGE queues (the serialized
descriptor-heavy loads used to stall the first ~35us), and each image is
DMA'd out per sample instead of staging all 32.
"""

import math

import numpy as np
import orjson

import bass_rust
import concourse.bass as bass
import concourse.mybir as mybir
import concourse.tile as tile
from concourse.bass_utils import run_bass_kernel_spmd

B, L, N, W = 256, 160, 30, 60
NCORES = 8
BC = B // NCORES          # samples per core
ALPHA = 2e-4
KEXP = 1.0 / (W * W * ALPHA)          # exponent scale in cell units: 1/0.72
SDERF = math.sqrt(KEXP)               # Derivative_Erf input scale
DERF_FIX = math.pi / 4.0              # undo (2/sqrt(pi))^2 from Derivative_Erf
CHUNKS = 40                           # 4 curves x 30 samples per chunk
PTS = 128                             # chunk partition dim: p = 32*lg + n
CW = 60                               # width of one chunk's band (= W)
R_HOLE = -60.0                        # r for dead rows -> d in [60,119] -> g=0

LAST_RESULTS = None  # test harness reads profiling info from here


def _basis_T() -> np.ndarray:
    t = np.arange(N, dtype=np.float32) / np.float32(N)
    t = 2 * t**3 - 3 * t**2 + 2 * t
    t_3_0 = t**3
    t_2_1 = t**2 - t_3_0
    t_1_2 = t_3_0 - 2 * t**2 + t
    t_0_3 = (1 - t) ** 3
    return np.stack([t_3_0, 3 * t_2_1, 3 * t_1_2, t_0_3], axis=1).astype(np.float32)


def _legalize_waits(nc, max_waits: int = 1):
    """Walrus rejects engine instructions carrying more than ~1 sync wait
    ("Too many sync wait commands").  Hoist excess waits onto same-engine
    Drain instructions inserted immediately before the offender."""
    js = orjson.loads(mybir.module_to_json_bytes(nc.m))
    ctr = 0
    for f in js["functions"]:
        for bb in f["blocks"]:
            out = []
            changed = False
            for inst in bb["instructions"]:
                si = inst.get("sync_info")
                waits = si.get("on_wait") if si else None
                if waits and len(waits) > max_waits:
                    keep = waits[:max_waits]
                    for w in waits[max_waits:]:
                        ctr += 1
                        out.append({
                            "debug": inst.get("debug", 0),
                            "engine": inst["engine"],
                            "ins": [], "outs": [],
                            "name": f"waitfix-{ctr}",
                            "opcode": "Drain",
                            "sync_info": {"on_update": [], "on_wait": [w]},
                        })
                    si["on_wait"] = keep
                    changed = True
                out.append(inst)
            if changed:
                bb["instructions"] = out
    if ctr:
        nc.m = bass_rust.module_from_json_bytes(orjson.dumps(js))
    return ctr


def build_program(legalize: bool = True):
    f32 = mybir.dt.float32
    f16 = mybir.dt.float16

    nc = bass.Bass("TRN2", target_bir_lowering=False, debug=False)

    x_t = nc.dram_tensor("x", [BC, L, 8], f32, kind="ExternalInput")
    y_t = nc.dram_tensor("y", [BC, W, W], f32, kind="ExternalOutput")

    # (5, 32) stationary operand: r[m] = sum_k TscT[k,m]*ctrl[k] = 60*X for
    # m<30; row 4 contracts against a constant-ones row so the two dead
    # partitions of each 32-strip get r = R_HOLE (-> g = 0) with no memset.
    tsc_np = np.zeros((5, 32), dtype=np.float32)
    tsc_np[:4, :N] = (W * _basis_T()).T
    tsc_np[4, N:] = R_HOLE
    tsc_d = nc.inline_tensor(tsc_np, name="tscT")
    iota_np = np.tile(np.arange(CW, dtype=np.float16), (PTS, 1))  # (128, 60)
    iota_d = nc.inline_tensor(iota_np, name="iota60")

    with tile.TileContext(nc) as tc, tc.tile_pool(name="const", bufs=1) as cpool, \
            tc.tile_pool(name="ctrl", bufs=1) as ctrl_pool, \
            tc.tile_pool(name="rwork", bufs=2) as rpool, \
            tc.tile_pool(name="band", bufs=4) as band_pool, \
            tc.tile_pool(name="rpsum", bufs=2, space="PSUM") as rps_pool, \
            tc.tile_pool(name="imgpsum", bufs=2, space="PSUM") as img_pool:

        # Prologue: DMA loads land in staging tiles; DVE copies them into the
        # tiles PE reads (PE LDWEIGHTS tolerates very few sync waits).
        tsc0 = cpool.tile([5, 32], f32, tag="tsc0")
        nc.sync.dma_start(tsc0[:], tsc_d.ap())
        tsc = cpool.tile([5, 32], f32, tag="tsc")
        nc.vector.tensor_copy(tsc[:], tsc0[:])
        iot = cpool.tile([PTS, CW], f16, tag="iota")
        nc.sync.dma_start(iot[:], iota_d.ap())

        # control points: partition k (4) + ones row (4), free = (b, l, coord).
        # Loaded in groups of 4 samples alternating between the two HWDGE
        # queues (qSP/qAct) so the descriptor-heavy strided loads overlap.
        GRP = 4
        ct = ctrl_pool.tile([5, BC * 2 * L], f32, tag="ct")
        nc.vector.memset(ct[:], 1.0)          # row 4 stays all-ones
        gsz = GRP * 2 * L
        for g in range(BC // GRP):
            ct0 = rpool.tile([4, gsz], f32, tag="ct0")
            eng = nc.sync if g % 2 == 0 else nc.scalar
            eng.dma_start(
                ct0[:].rearrange("k (b l c) -> k b l c", b=GRP, c=2),
                x_t.ap()[g * GRP : (g + 1) * GRP]
                .rearrange("b l (k c) -> k b l c", k=4),
            )
            nc.vector.tensor_copy(ct[0:4, g * gsz : (g + 1) * gsz], ct0[:])
        ct_v = ct[:].rearrange("k (b c g co) -> k b c g co", b=BC, c=CHUNKS, co=2)

        for b in range(BC):
            # ---- r = 60 * point coords, layout [p=(lg,n), (chunk, coord)] ----
            r_ps = rps_pool.tile([PTS, 2 * CHUNKS], f32, tag="rps")
            for lg in range(4):
                nc.tensor.matmul(
                    r_ps[32 * lg : 32 * lg + 32, :],
                    lhsT=tsc[:],
                    rhs=ct_v[:, b : b + 1, :, lg : lg + 1, :],
                    start=True,
                    stop=True,
                    tile_position=(0, 32 * lg),
                )
            # PSUM -> SBUF on the Scalar engine; the DVE is the pace-setter.
            r_sb = rpool.tile([PTS, 2 * CHUNKS], f32, tag="rsb")
            nc.scalar.copy(r_sb[:], r_ps[:])

            # ---- banded distance + Gaussian, fp16.  Free-dim layout is
            # (chunk, side, cell): ONE batched tensor_tensor with broadcast
            # APs computes every distance of the sample.
            dd = band_pool.tile([PTS, 2 * CHUNKS * CW], f16, tag="dd")
            nc.vector.tensor_tensor(
                dd[:].rearrange("p (cs w) -> p cs w", w=CW),
                iot[:].rearrange("p (o w) -> p o w", o=1).broadcast_to(
                    [PTS, 2 * CHUNKS, CW]
                ),
                r_sb[:].rearrange("p (cs o) -> p cs o", o=1).broadcast_to(
                    [PTS, 2 * CHUNKS, CW]
                ),
                mybir.AluOpType.subtract,
            )
            gg = band_pool.tile([PTS, 2 * CHUNKS * CW], f16, tag="gg")
            nc.scalar.activation(
                gg[:], dd[:],
                mybir.ActivationFunctionType.Derivative_Erf,
                bias=0.0, scale=SDERF,
            )

            # ---- image accumulation: sum_c GxT_c^T @ GyT_c ----
            img = img_pool.tile([W, W], f32, tag="img")
            for c in range(CHUNKS):
                nc.tensor.matmul(
                    img[:],
                    lhsT=gg[:, 2 * CW * c : 2 * CW * c + W],
                    rhs=gg[:, 2 * CW * c + CW : 2 * CW * c + CW + W],
                    start=(c == 0),
                    stop=(c == CHUNKS - 1),
                )

            # ---- min(scale*img, 1) -> staging -> per-sample DMA out ----
            osb = rpool.tile([W, W], f32, tag="osb")
            nc.vector.tensor_scalar(
                osb[:], img[:], DERF_FIX, 1.0,
                mybir.AluOpType.mult, mybir.AluOpType.min,
            )
            nc.sync.dma_start(y_t.ap()[b : b + 1], osb[:])

    if legalize:
        _legalize_waits(nc)
    return nc


_PROGRAM = None


def kernel(x: np.ndarray, _trace: bool = False) -> np.ndarray:
    global _PROGRAM, LAST_RESULTS
    assert x.shape == (B, L, 8) and x.dtype == np.float32, (x.shape, x.dtype)
    if _PROGRAM is None:
        _PROGRAM = build_program()
    nc = _PROGRAM
    shards = np.split(np.ascontiguousarray(x), NCORES, axis=0)
    in_maps = [{"x": s} for s in shards]
    res = run_bass_kernel_spmd(nc, in_maps, list(range(NCORES)), trace=_trace)
    LAST_RESULTS = res
    return np.concatenate([res.results[i]["y"] for i in range(NCORES)], axis=0)
